# revision 20
# baseline (speedup 1.0000x reference)
"""Causal self-attention Trainium2 kernel (B=8, T=2048, C=256, H=4).

Sharding: batch B=8 across the 8 NeuronCores (data parallel, no collectives).
Each core computes one batch element end-to-end:
  qkv = x @ W_attn ; per-head causal softmax(q k^T / sqrt(hs)) @ v ; @ W_proj

Layout strategy (per core):
  - x [T,C] is DMA'd in, transposed on the tensor engine to xT [C,T] (bf16).
  - qT,kT [C_qk, T] computed transposed (feature rows on partitions), with
    softmax_scale*log2(e) folded into qT so scores are in log2 units.
  - v [T, C_v] computed untransposed.
  - S^T tiles (k on partitions, q on free dim) = kT_tile.T @ qT_block; two
    heads packed concurrently in the PE array (K=64 row groups 0/64).
  - exp2 via ScalarE activation(Exp, scale=ln2) over multi-bank PSUM groups.
  - causal mask on diagonal 128x128 blocks via gpsimd affine_select on P.
  - O^T += V_tile.T @ P (two heads col-packed, output partitions 0-63/64-127),
    row sums += ones.T @ P (M=1 matmuls at col positions 0/64).
  - normalization folded into the PSUM->SBUF drain: O^T * broadcast(1/sums).
  - proj: z = Y @ W_proj from the stacked Y^T, DMA out.
"""

import sys

if "/opt/trn_rl_repo" not in sys.path:
    sys.path.insert(0, "/opt/trn_rl_repo")

import numpy as np

import concourse.bass as bass
import concourse.mybir as mybir
from concourse import bacc
from concourse.masks import make_identity
from concourse.tile import TileContext

B, T, C = 8, 2048, 256
H, HS = 4, 64
NT = T // 128            # 16 token tiles
NQB = T // 512           # 4 q blocks of 512
F32 = mybir.dt.float32
BF16 = mybir.dt.bfloat16
LOG2E = 1.4426950408889634
LN2 = 0.6931471805599453
QSCALE = LOG2E / 8.0     # softmax scale 1/sqrt(hs) in log2 units
EXP_GROUP = 3            # S tiles per exp2 activation (3 psum banks)

_cached_nc = None


def _build(dbg=False):
    nc = bacc.Bacc("TRN2", target_bir_lowering=False, debug=False)
    x_d = nc.declare_dram_parameter("x", [T, C], F32, isOutput=False)
    wa_d = nc.declare_dram_parameter("W_attn", [C, 3 * C], F32, isOutput=False)
    wp_d = nc.declare_dram_parameter("W_proj", [C, C], F32, isOutput=False)
    y_d = nc.declare_dram_parameter("y", [T, C], F32, isOutput=True)
    if dbg:
        dbg_p = nc.declare_dram_parameter("dbg_p", [128, 4096], F32, isOutput=True)
        dbg_o = nc.declare_dram_parameter("dbg_o", [128, 512], F32, isOutput=True)
        dbg_s = nc.declare_dram_parameter("dbg_s", [128, 512], F32, isOutput=True)

    with TileContext(nc) as tc:
        sb = tc.alloc_tile_pool(name="sb", bufs=1)
        # persistent SBUF tensors
        x_sb = sb.tile([128, NT * 256], F32, name="x_sb")          # [t128, (n c)]
        xT = sb.tile([128, 2 * T], BF16, name="xT")                # [c128, (kc t)]
        qT = sb.tile([128, 2 * T], BF16, name="qT")                # [feat128, (fh t)]
        kT = sb.tile([128, 2 * T], BF16, name="kT")
        v_sb = sb.tile([128, NT * 256], BF16, name="v_sb")         # [t128, (n c)]
        yT = sb.tile([128, 2 * T], BF16, name="yT")                # [feat128, (fh t)]
        wa_f = sb.tile([128, 2 * 768], F32, name="wa_f")
        wa_b = sb.tile([128, 2 * 768], BF16, name="wa_b")
        wp_f = sb.tile([128, 2 * 256], F32, name="wp_f")
        wp_b = sb.tile([128, 2 * 256], BF16, name="wp_b")
        ident = sb.tile([128, 128], F32, name="ident")
        ones_b = sb.tile([128, 1], BF16, name="ones_b")
        ones_f = sb.tile([128, 64], F32, name="ones_f")
        zeros_b = sb.tile([128, 512], BF16, name="zeros_b")

        make_identity(nc, ident)
        nc.gpsimd.memset(ones_b, 1.0)
        nc.gpsimd.memset(ones_f, 1.0)
        nc.gpsimd.memset(zeros_b, 0.0)

        # ---- load inputs ----
        nc.sync.dma_start(
            x_sb.rearrange("p (n c) -> p n c", n=NT),
            x_d[:].rearrange("(n p) c -> p n c", p=128),
        )
        nc.sync.dma_start(
            wa_f.rearrange("p (k m) -> p k m", k=2),
            wa_d[:].rearrange("(k p) m -> p k m", p=128),
        )
        nc.sync.dma_start(
            wp_f.rearrange("p (k m) -> p k m", k=2),
            wp_d[:].rearrange("(k p) m -> p k m", p=128),
        )
        nc.vector.tensor_copy(wa_b[:], wa_f[:])
        nc.vector.tensor_copy(wp_b[:], wp_f[:])

        # ---- setup phase: transpose x, compute qT/kT/v ----
        with tc.tile_pool(name="pset", bufs=1, space="PSUM") as pset:
            # x transpose: 32 [128,128] PE transposes, batched 4 per psum bank
            for kc in range(2):
                for ng in range(4):
                    tp = pset.tile([128, 512], F32, tag="tp", bufs=2)
                    for j in range(4):
                        n = ng * 4 + j
                        nc.tensor.transpose(
                            tp[:, j * 128:(j + 1) * 128],
                            x_sb[:, n * 256 + kc * 128: n * 256 + kc * 128 + 128],
                            ident,
                        )
                    nc.vector.tensor_copy(
                        xT[:, kc * T + ng * 512: kc * T + ng * 512 + 512], tp[:]
                    )
            # qT, kT: feature-half fh covers heads (2fh, 2fh+1)
            for fh in range(2):
                for nb in range(NQB):
                    rhs = xT[:, 0 * T + nb * 512: 0 * T + nb * 512 + 512]
                    rhs1 = xT[:, 1 * T + nb * 512: 1 * T + nb * 512 + 512]
                    ps_q = pset.tile([128, 512], F32, tag="mm", bufs=2)
                    nc.tensor.matmul(
                        ps_q, wa_b[:, 0 * 768 + fh * 128: 0 * 768 + fh * 128 + 128],
                        rhs, start=True, stop=False,
                    )
                    nc.tensor.matmul(
                        ps_q, wa_b[:, 1 * 768 + fh * 128: 1 * 768 + fh * 128 + 128],
                        rhs1, start=False, stop=True,
                    )
                    nc.vector.tensor_scalar_mul(
                        qT[:, fh * T + nb * 512: fh * T + nb * 512 + 512], ps_q, QSCALE
                    )
                    ps_k = pset.tile([128, 512], F32, tag="mm", bufs=2)
                    nc.tensor.matmul(
                        ps_k,
                        wa_b[:, 0 * 768 + 256 + fh * 128: 0 * 768 + 256 + fh * 128 + 128],
                        rhs, start=True, stop=False,
                    )
                    nc.tensor.matmul(
                        ps_k,
                        wa_b[:, 1 * 768 + 256 + fh * 128: 1 * 768 + 256 + fh * 128 + 128],
                        rhs1, start=False, stop=True,
                    )
                    nc.vector.tensor_copy(
                        kT[:, fh * T + nb * 512: fh * T + nb * 512 + 512], ps_k
                    )
            # v (untransposed): v[t, c] for t-tile n
            for n in range(NT):
                ps_v = pset.tile([128, 256], F32, tag="mm", bufs=2)
                for kc in range(2):
                    nc.tensor.matmul(
                        ps_v,
                        xT[:, kc * T + n * 128: kc * T + n * 128 + 128],
                        wa_b[:, kc * 768 + 512: kc * 768 + 768],
                        start=(kc == 0),
                        stop=(kc == 1),
                    )
                nc.vector.tensor_copy(v_sb[:, n * 256: n * 256 + 256], ps_v)

        # ---- attention ----
        with tc.tile_pool(name="pat", bufs=1, space="PSUM") as pat:
            for hp in range(2):          # head pair: global heads (2hp, 2hp+1)
                for tqb in range(NQB):
                    ntk = 4 * (tqb + 1)
                    tiles = [(h, tk) for tk in range(ntk) for h in range(2)]
                    groups = [
                        tiles[i: i + EXP_GROUP]
                        for i in range(0, len(tiles), EXP_GROUP)
                    ]
                    oacc = pat.tile([128, 512], F32, tag="oacc", bufs=1)
                    sums = pat.tile([128, 512], F32, tag="sums", bufs=1)
                    # zero the whole accumulator banks first (sets every
                    # has_written bit, so the interleaved per-region matmuls
                    # below are order-free)
                    nc.tensor.matmul(
                        oacc[:], zeros_b[0:1, 0:128], zeros_b[0:1, :],
                        start=True, stop=False, skip_group_check=True,
                    )
                    nc.tensor.matmul(
                        sums[:], zeros_b[0:1, 0:128], zeros_b[0:1, :],
                        start=True, stop=False, skip_group_check=True,
                    )
                    n_pv = 0
                    dbg_col = 0
                    for grp in groups:
                        gw = 512 * len(grp)
                        sg = pat.tile([128, gw], F32, tag="sg", bufs=2)
                        pg = sb.tile([128, gw], BF16, tag="P", bufs=3, name="pg")
                        for j, (h, tk) in enumerate(grp):
                            nc.tensor.matmul(
                                sg[:, j * 512:(j + 1) * 512],
                                kT[64 * h: 64 * h + 64,
                                   hp * T + tk * 128: hp * T + tk * 128 + 128],
                                qT[64 * h: 64 * h + 64,
                                   hp * T + tqb * 512: hp * T + tqb * 512 + 512],
                                start=True, stop=True,
                            )
                        # P = 2^(S^T)  (scores already in log2 units)
                        nc.scalar.activation(
                            pg[:], sg[:], mybir.ActivationFunctionType.Exp, scale=LN2
                        )
                        for j, (h, tk) in enumerate(grp):
                            if tk >= 4 * tqb:  # diagonal tile: zero where q < k
                                off = (tk - 4 * tqb) * 128
                                w = off + 128  # cols beyond are always valid
                                nc.gpsimd.affine_select(
                                    out=pg[:, j * 512: j * 512 + w],
                                    in_=pg[:, j * 512: j * 512 + w],
                                    compare_op=mybir.AluOpType.is_ge,
                                    fill=0.0,
                                    base=-off,
                                    pattern=[[1, w]],
                                    channel_multiplier=-1,
                                )
                            gh = 2 * hp + h
                            n_pv += 1
                            nc.tensor.matmul(
                                oacc[64 * h: 64 * h + 64, :],
                                v_sb[:, tk * 256 + gh * 64: tk * 256 + gh * 64 + 64],
                                pg[:, j * 512:(j + 1) * 512],
                                start=False, stop=(n_pv == len(tiles)),
                                skip_group_check=True,
                            )
                            nc.tensor.matmul(
                                sums[64 * h: 64 * h + 1, :],
                                ones_b[:],
                                pg[:, j * 512:(j + 1) * 512],
                                start=False, stop=(n_pv == len(tiles)),
                                skip_group_check=True,
                            )
                        if dbg and hp == 0 and tqb == 0:
                            dpt = sb.tile([128, 1536], F32, tag="dbgp", bufs=2, name="dpt")
                            nc.vector.tensor_copy(dpt[:, :gw], pg[:])
                            nc.sync.dma_start(dbg_p[:, dbg_col: dbg_col + gw], dpt[:, :gw])
                            dbg_col += gw
                    if dbg and hp == 0 and tqb == 0:
                        dtile = sb.tile([128, 512], F32, tag="dbgt", bufs=2, name="dtile")
                        nc.vector.tensor_copy(dtile, oacc[:])
                        nc.sync.dma_start(dbg_o[:], dtile)
                        dtile2 = sb.tile([128, 512], F32, tag="dbgt", bufs=2, name="dtile2")
                        nc.vector.tensor_copy(dtile2, sums[:])
                        nc.sync.dma_start(dbg_s[:], dtile2)
                    recip = sb.tile([128, 512], F32, tag="recip", bufs=2, name="recip")
                    bcast = sb.tile([128, 512], F32, tag="bcast", bufs=2, name="bcast")
                    nc.vector.reciprocal(recip[0:65, :], sums[0:65, :])
                    # broadcast 1/sums across the 64 head rows via PE outer product
                    bcps = pat.tile([128, 512], F32, tag="sg", bufs=2, name="bcps")
                    nc.tensor.matmul(
                        bcps[0:64, :], ones_f[0:1, :], recip[0:1, :],
                        start=True, stop=True,
                    )
                    nc.tensor.matmul(
                        bcps[64:128, :], ones_f[64:65, :], recip[64:65, :],
                        start=True, stop=True,
                    )
                    nc.vector.tensor_copy(bcast[:], bcps[:])
                    nc.vector.tensor_mul(
                        yT[:, hp * T + tqb * 512: hp * T + tqb * 512 + 512],
                        oacc[:], bcast[:],
                    )

        # ---- output projection ----
        with tc.tile_pool(name="ppr", bufs=1, space="PSUM") as ppr:
            for n in range(NT):
                psz = ppr.tile([128, 256], F32, tag="mm2", bufs=3)
                for fh in range(2):
                    nc.tensor.matmul(
                        psz,
                        yT[:, fh * T + n * 128: fh * T + n * 128 + 128],
                        wp_b[:, fh * 256: fh * 256 + 256],
                        start=(fh == 0),
                        stop=(fh == 1),
                    )
                z_sb = sb.tile([128, 256], F32, tag="z", bufs=3, name="z_sb")
                nc.vector.tensor_copy(z_sb, psz)
                nc.sync.dma_start(
                    y_d[:].rearrange("(n p) c -> p n c", p=128)[:, n: n + 1],
                    z_sb.rearrange("p (n c) -> p n c", n=1),
                )
        sb.release()
    nc.compile()
    return nc


def _get_nc():
    global _cached_nc
    if _cached_nc is None:
        _cached_nc = _build()
    return _cached_nc


def kernel(**inputs):
    from concourse.bass_utils import run_bass_kernel_spmd

    x = np.ascontiguousarray(np.asarray(inputs["x"], dtype=np.float32))
    wa = np.ascontiguousarray(np.asarray(inputs["W_attn"], dtype=np.float32))
    wp = np.ascontiguousarray(np.asarray(inputs["W_proj"], dtype=np.float32))
    nc = _get_nc()
    in_maps = [
        {"x": np.ascontiguousarray(x[b]), "W_attn": wa, "W_proj": wp}
        for b in range(B)
    ]
    res = run_bass_kernel_spmd(nc, in_maps, core_ids=list(range(B)))
    return np.stack([res.results[b]["y"] for b in range(B)], axis=0)


# revision 21
# speedup vs baseline: 1.1506x; 1.1506x over previous
"""Causal self-attention Trainium2 kernel (B=8, T=2048, C=256, H=4).

Sharding: batch B=8 across the 8 NeuronCores (data parallel, no collectives).
Each core computes one batch element end-to-end:
  qkv = x @ W_attn ; per-head causal softmax(q k^T / sqrt(hs)) @ v ; @ W_proj

Layout strategy (per core):
  - x [T,C] is DMA'd in, transposed on the tensor engine to xT [C,T] (bf16).
  - qT,kT [C_qk, T] computed transposed (feature rows on partitions), with
    softmax_scale*log2(e) folded into qT so scores are in log2 units.
  - v [T, C_v] computed untransposed.
  - S^T tiles (k on partitions, q on free dim) = kT_tile.T @ qT_block; two
    heads packed concurrently in the PE array (K=64 row groups 0/64).
  - exp2 via ScalarE activation(Exp, scale=ln2) over multi-bank PSUM groups.
  - causal mask on diagonal 128x128 blocks via gpsimd affine_select on P.
  - O^T += V_tile.T @ P (two heads col-packed, output partitions 0-63/64-127),
    row sums += ones.T @ P (M=1 matmuls at col positions 0/64).
  - normalization folded into the PSUM->SBUF drain: O^T * broadcast(1/sums).
  - proj: z = Y @ W_proj from the stacked Y^T, DMA out.
"""

import sys

if "/opt/trn_rl_repo" not in sys.path:
    sys.path.insert(0, "/opt/trn_rl_repo")

import numpy as np

import concourse.bass as bass
import concourse.mybir as mybir
from concourse import bacc
from concourse.masks import make_identity
from concourse.tile import TileContext

B, T, C = 8, 2048, 256
H, HS = 4, 64
NT = T // 128            # 16 token tiles
NQB = T // 512           # 4 q blocks of 512
F32 = mybir.dt.float32
BF16 = mybir.dt.bfloat16
LOG2E = 1.4426950408889634
LN2 = 0.6931471805599453
QSCALE = LOG2E / 8.0     # softmax scale 1/sqrt(hs) in log2 units
EXP_GROUP = 3            # S tiles per exp2 activation (3 psum banks)

_cached_nc = None


def _build(dbg=False):
    nc = bacc.Bacc("TRN2", target_bir_lowering=False, debug=False)
    x_d = nc.declare_dram_parameter("x", [T, C], F32, isOutput=False)
    wa_d = nc.declare_dram_parameter("W_attn", [C, 3 * C], F32, isOutput=False)
    wp_d = nc.declare_dram_parameter("W_proj", [C, C], F32, isOutput=False)
    y_d = nc.declare_dram_parameter("y", [T, C], F32, isOutput=True)
    if dbg:
        dbg_p = nc.declare_dram_parameter("dbg_p", [128, 4096], F32, isOutput=True)
        dbg_o = nc.declare_dram_parameter("dbg_o", [128, 512], F32, isOutput=True)
        dbg_s = nc.declare_dram_parameter("dbg_s", [128, 512], F32, isOutput=True)

    with TileContext(nc) as tc:
        sb = tc.alloc_tile_pool(name="sb", bufs=1)
        # persistent SBUF tensors
        x_sb = sb.tile([128, NT * 256], F32, name="x_sb")          # [t128, (n c)]
        xT = sb.tile([128, 2 * T], BF16, name="xT")                # [c128, (kc t)]
        qT = sb.tile([128, 2 * T], BF16, name="qT")                # [feat128, (fh t)]
        kT = sb.tile([128, 2 * T], BF16, name="kT")
        v_sb = sb.tile([128, NT * 256], BF16, name="v_sb")         # [t128, (n c)]
        yT = sb.tile([128, 2 * T], BF16, name="yT")                # [feat128, (fh t)]
        wa_f = sb.tile([128, 2 * 768], F32, name="wa_f")
        wa_b = sb.tile([128, 2 * 768], BF16, name="wa_b")
        wp_f = sb.tile([128, 2 * 256], F32, name="wp_f")
        wp_b = sb.tile([128, 2 * 256], BF16, name="wp_b")
        ident = sb.tile([128, 128], F32, name="ident")
        ones_b = sb.tile([128, 1], BF16, name="ones_b")
        ones_f = sb.tile([128, 64], F32, name="ones_f")
        zeros_b = sb.tile([128, 512], BF16, name="zeros_b")

        make_identity(nc, ident)
        nc.gpsimd.memset(ones_b, 1.0)
        nc.gpsimd.memset(ones_f, 1.0)
        nc.gpsimd.memset(zeros_b, 0.0)

        # ---- load inputs ----
        nc.sync.dma_start(
            x_sb.rearrange("p (n c) -> p n c", n=NT),
            x_d[:].rearrange("(n p) c -> p n c", p=128),
        )
        nc.sync.dma_start(
            wa_f.rearrange("p (k m) -> p k m", k=2),
            wa_d[:].rearrange("(k p) m -> p k m", p=128),
        )
        nc.sync.dma_start(
            wp_f.rearrange("p (k m) -> p k m", k=2),
            wp_d[:].rearrange("(k p) m -> p k m", p=128),
        )
        nc.vector.tensor_copy(wa_b[:], wa_f[:])
        nc.vector.tensor_copy(wp_b[:], wp_f[:])

        # ---- setup phase: transpose x, compute qT/kT/v ----
        with tc.tile_pool(name="pset", bufs=1, space="PSUM") as pset:
            # x transpose: 32 [128,128] PE transposes, batched 4 per psum bank
            for kc in range(2):
                for ng in range(4):
                    tp = pset.tile([128, 512], F32, tag="tp", bufs=2)
                    for j in range(4):
                        n = ng * 4 + j
                        nc.tensor.transpose(
                            tp[:, j * 128:(j + 1) * 128],
                            x_sb[:, n * 256 + kc * 128: n * 256 + kc * 128 + 128],
                            ident,
                        )
                    nc.vector.tensor_copy(
                        xT[:, kc * T + ng * 512: kc * T + ng * 512 + 512], tp[:]
                    )
            # qT, kT: feature-half fh covers heads (2fh, 2fh+1)
            for fh in range(2):
                for nb in range(NQB):
                    rhs = xT[:, 0 * T + nb * 512: 0 * T + nb * 512 + 512]
                    rhs1 = xT[:, 1 * T + nb * 512: 1 * T + nb * 512 + 512]
                    ps_q = pset.tile([128, 512], F32, tag="mm", bufs=2)
                    nc.tensor.matmul(
                        ps_q, wa_b[:, 0 * 768 + fh * 128: 0 * 768 + fh * 128 + 128],
                        rhs, start=True, stop=False,
                    )
                    nc.tensor.matmul(
                        ps_q, wa_b[:, 1 * 768 + fh * 128: 1 * 768 + fh * 128 + 128],
                        rhs1, start=False, stop=True,
                    )
                    nc.vector.tensor_scalar_mul(
                        qT[:, fh * T + nb * 512: fh * T + nb * 512 + 512], ps_q, QSCALE
                    )
                    ps_k = pset.tile([128, 512], F32, tag="mm", bufs=2)
                    nc.tensor.matmul(
                        ps_k,
                        wa_b[:, 0 * 768 + 256 + fh * 128: 0 * 768 + 256 + fh * 128 + 128],
                        rhs, start=True, stop=False,
                    )
                    nc.tensor.matmul(
                        ps_k,
                        wa_b[:, 1 * 768 + 256 + fh * 128: 1 * 768 + 256 + fh * 128 + 128],
                        rhs1, start=False, stop=True,
                    )
                    nc.vector.tensor_copy(
                        kT[:, fh * T + nb * 512: fh * T + nb * 512 + 512], ps_k
                    )
            # v (untransposed): v[t, c] for t-tile n
            for n in range(NT):
                ps_v = pset.tile([128, 256], F32, tag="mm", bufs=2)
                for kc in range(2):
                    nc.tensor.matmul(
                        ps_v,
                        xT[:, kc * T + n * 128: kc * T + n * 128 + 128],
                        wa_b[:, kc * 768 + 512: kc * 768 + 768],
                        start=(kc == 0),
                        stop=(kc == 1),
                    )
                nc.vector.tensor_copy(v_sb[:, n * 256: n * 256 + 256], ps_v)

        # ---- attention ----
        with tc.tile_pool(name="pat", bufs=1, space="PSUM") as pat:
            for hp in range(2):          # head pair: global heads (2hp, 2hp+1)
                for tqb in range(NQB):
                    ntk = 4 * (tqb + 1)
                    tiles = [(h, tk) for tk in range(ntk) for h in range(2)]
                    groups = [
                        tiles[i: i + EXP_GROUP]
                        for i in range(0, len(tiles), EXP_GROUP)
                    ]
                    oacc = pat.tile([128, 512], F32, tag="oacc", bufs=1)
                    sums = pat.tile([128, 512], F32, tag="sums", bufs=1)
                    # zero the whole accumulator banks first (sets every
                    # has_written bit, so the interleaved per-region matmuls
                    # below are order-free)
                    nc.tensor.matmul(
                        oacc[:], zeros_b[0:1, 0:128], zeros_b[0:1, :],
                        start=True, stop=False, skip_group_check=True,
                    )
                    nc.tensor.matmul(
                        sums[:], zeros_b[0:1, 0:128], zeros_b[0:1, :],
                        start=True, stop=False, skip_group_check=True,
                    )
                    n_pv = 0
                    dbg_col = 0
                    for grp in groups:
                        gw = 512 * len(grp)
                        sg = pat.tile([128, gw], F32, tag="sg", bufs=2)
                        pg = sb.tile([128, gw], BF16, tag="P", bufs=3, name="pg")
                        for j, (h, tk) in enumerate(grp):
                            nc.tensor.matmul(
                                sg[:, j * 512:(j + 1) * 512],
                                kT[64 * h: 64 * h + 64,
                                   hp * T + tk * 128: hp * T + tk * 128 + 128],
                                qT[64 * h: 64 * h + 64,
                                   hp * T + tqb * 512: hp * T + tqb * 512 + 512],
                                start=True, stop=True,
                            )
                        # P = 2^(S^T)  (scores already in log2 units)
                        nc.scalar.activation(
                            pg[:], sg[:], mybir.ActivationFunctionType.Exp, scale=LN2
                        )
                        for j, (h, tk) in enumerate(grp):
                            if tk >= 4 * tqb:  # diagonal tile: zero where q < k
                                off = (tk - 4 * tqb) * 128
                                w = off + 128  # cols beyond are always valid
                                nc.gpsimd.affine_select(
                                    out=pg[:, j * 512: j * 512 + w],
                                    in_=pg[:, j * 512: j * 512 + w],
                                    compare_op=mybir.AluOpType.is_ge,
                                    fill=0.0,
                                    base=-off,
                                    pattern=[[1, w]],
                                    channel_multiplier=-1,
                                )
                        # PV first (head pairs adjacent -> concurrent in the
                        # array via col groups), then the M=1 sums pairs
                        for j, (h, tk) in enumerate(grp):
                            gh = 2 * hp + h
                            n_pv += 1
                            nc.tensor.matmul(
                                oacc[64 * h: 64 * h + 64, :],
                                v_sb[:, tk * 256 + gh * 64: tk * 256 + gh * 64 + 64],
                                pg[:, j * 512:(j + 1) * 512],
                                start=False, stop=(n_pv == len(tiles)),
                                skip_group_check=True,
                            )
                        for j, (h, tk) in enumerate(grp):
                            nc.tensor.matmul(
                                sums[64 * h: 64 * h + 1, :],
                                ones_b[:],
                                pg[:, j * 512:(j + 1) * 512],
                                start=False, stop=(n_pv == len(tiles) and j == len(grp) - 1),
                                skip_group_check=True,
                            )
                        if dbg and hp == 0 and tqb == 0:
                            dpt = sb.tile([128, 1536], F32, tag="dbgp", bufs=2, name="dpt")
                            nc.vector.tensor_copy(dpt[:, :gw], pg[:])
                            nc.sync.dma_start(dbg_p[:, dbg_col: dbg_col + gw], dpt[:, :gw])
                            dbg_col += gw
                    if dbg and hp == 0 and tqb == 0:
                        dtile = sb.tile([128, 512], F32, tag="dbgt", bufs=2, name="dtile")
                        nc.vector.tensor_copy(dtile, oacc[:])
                        nc.sync.dma_start(dbg_o[:], dtile)
                        dtile2 = sb.tile([128, 512], F32, tag="dbgt", bufs=2, name="dtile2")
                        nc.vector.tensor_copy(dtile2, sums[:])
                        nc.sync.dma_start(dbg_s[:], dtile2)
                    recip = sb.tile([128, 512], F32, tag="recip", bufs=2, name="recip")
                    bcast = sb.tile([128, 512], F32, tag="bcast", bufs=2, name="bcast")
                    nc.vector.reciprocal(recip[0:65, :], sums[0:65, :])
                    # broadcast 1/sums across the 64 head rows via PE outer product
                    bcps = pat.tile([128, 512], F32, tag="sg", bufs=2, name="bcps")
                    nc.tensor.matmul(
                        bcps[0:64, :], ones_f[0:1, :], recip[0:1, :],
                        start=True, stop=True,
                    )
                    nc.tensor.matmul(
                        bcps[64:128, :], ones_f[64:65, :], recip[64:65, :],
                        start=True, stop=True,
                    )
                    nc.vector.tensor_copy(bcast[:], bcps[:])
                    nc.vector.tensor_mul(
                        yT[:, hp * T + tqb * 512: hp * T + tqb * 512 + 512],
                        oacc[:], bcast[:],
                    )

        # ---- output projection ----
        with tc.tile_pool(name="ppr", bufs=1, space="PSUM") as ppr:
            for n in range(NT):
                psz = ppr.tile([128, 256], F32, tag="mm2", bufs=3)
                for fh in range(2):
                    nc.tensor.matmul(
                        psz,
                        yT[:, fh * T + n * 128: fh * T + n * 128 + 128],
                        wp_b[:, fh * 256: fh * 256 + 256],
                        start=(fh == 0),
                        stop=(fh == 1),
                    )
                z_sb = sb.tile([128, 256], F32, tag="z", bufs=3, name="z_sb")
                nc.vector.tensor_copy(z_sb, psz)
                nc.sync.dma_start(
                    y_d[:].rearrange("(n p) c -> p n c", p=128)[:, n: n + 1],
                    z_sb.rearrange("p (n c) -> p n c", n=1),
                )
        sb.release()
    nc.compile()
    return nc


def _get_nc():
    global _cached_nc
    if _cached_nc is None:
        _cached_nc = _build()
    return _cached_nc


def kernel(**inputs):
    from concourse.bass_utils import run_bass_kernel_spmd

    x = np.ascontiguousarray(np.asarray(inputs["x"], dtype=np.float32))
    wa = np.ascontiguousarray(np.asarray(inputs["W_attn"], dtype=np.float32))
    wp = np.ascontiguousarray(np.asarray(inputs["W_proj"], dtype=np.float32))
    nc = _get_nc()
    in_maps = [
        {"x": np.ascontiguousarray(x[b]), "W_attn": wa, "W_proj": wp}
        for b in range(B)
    ]
    res = run_bass_kernel_spmd(nc, in_maps, core_ids=list(range(B)))
    return np.stack([res.results[b]["y"] for b in range(B)], axis=0)


# revision 27
# speedup vs baseline: 1.3446x; 1.1686x over previous
"""Causal self-attention Trainium2 kernel (B=8, T=2048, C=256, H=4).

Sharding: batch B=8 across the 8 NeuronCores (data parallel, no collectives).
Each core computes one batch element end-to-end:
  qkv = x @ W_attn ; per-head causal softmax(q k^T / sqrt(hs)) @ v ; @ W_proj

Layout strategy (per core):
  - x [T,C] is DMA'd in, transposed on the tensor engine to xT [C,T] (bf16).
  - qT,kT [C_qk, T] computed transposed (feature rows on partitions), with
    softmax_scale*log2(e) folded into qT so scores are in log2 units.
  - v [T, C_v] computed untransposed.
  - S^T tiles (k on partitions, q on free dim) = kT_tile.T @ qT_block; two
    heads packed concurrently in the PE array (K=64 row groups 0/64).
  - exp2 via ScalarE activation(Exp, scale=ln2) over multi-bank PSUM groups.
  - causal mask on diagonal 128x128 blocks via gpsimd affine_select on P.
  - O^T += V_tile.T @ P (two heads col-packed, output partitions 0-63/64-127),
    row sums += ones.T @ P (M=1 matmuls at col positions 0/64).
  - normalization folded into the PSUM->SBUF drain: O^T * broadcast(1/sums).
  - proj: z = Y @ W_proj from the stacked Y^T, DMA out.
"""

import sys

if "/opt/trn_rl_repo" not in sys.path:
    sys.path.insert(0, "/opt/trn_rl_repo")

import numpy as np

import concourse.bass as bass
import concourse.mybir as mybir
from concourse import bacc
from concourse.masks import make_identity
from concourse.tile import TileContext

B, T, C = 8, 2048, 256
H, HS = 4, 64
NT = T // 128            # 16 token tiles
NQB = T // 512           # 4 q blocks of 512
F32 = mybir.dt.float32
BF16 = mybir.dt.bfloat16
LOG2E = 1.4426950408889634
LN2 = 0.6931471805599453
QSCALE = LOG2E / 8.0     # softmax scale 1/sqrt(hs) in log2 units
EXP_GROUP = 3            # S tiles per exp2 activation (3 psum banks)

_cached_nc = None


def _build(dbg=False):
    nc = bacc.Bacc("TRN2", target_bir_lowering=False, debug=False)
    x_d = nc.declare_dram_parameter("x", [T, C], F32, isOutput=False)
    wa_d = nc.declare_dram_parameter("W_attn", [C, 3 * C], F32, isOutput=False)
    wp_d = nc.declare_dram_parameter("W_proj", [C, C], F32, isOutput=False)
    y_d = nc.declare_dram_parameter("y", [T, C], F32, isOutput=True)
    if dbg:
        dbg_p = nc.declare_dram_parameter("dbg_p", [128, 4096], F32, isOutput=True)
        dbg_o = nc.declare_dram_parameter("dbg_o", [128, 512], F32, isOutput=True)
        dbg_s = nc.declare_dram_parameter("dbg_s", [128, 512], F32, isOutput=True)

    with TileContext(nc) as tc:
        sb = tc.alloc_tile_pool(name="sb", bufs=1)
        # persistent SBUF tensors
        x_sb = sb.tile([128, NT * 256], F32, name="x_sb")          # [t128, (n c)]
        xT = sb.tile([128, 2 * T], BF16, name="xT")                # [c128, (kc t)]
        qT = sb.tile([128, 2 * T], BF16, name="qT")                # [feat128, (fh t)]
        kT = sb.tile([128, 2 * T], BF16, name="kT")
        v_sb = sb.tile([128, NT * 256], BF16, name="v_sb")         # [t128, (n c)]
        yT = sb.tile([128, 2 * T], BF16, name="yT")                # [feat128, (fh t)]
        wa_f = sb.tile([128, 2 * 768], F32, name="wa_f")
        wa_b = sb.tile([128, 2 * 768], BF16, name="wa_b")
        wp_f = sb.tile([128, 2 * 256], F32, name="wp_f")
        wp_b = sb.tile([128, 2 * 256], BF16, name="wp_b")
        ident = sb.tile([128, 128], F32, name="ident")
        ones_b = sb.tile([128, 1], BF16, name="ones_b")
        ones_f = sb.tile([128, 64], F32, name="ones_f")
        zeros_b = sb.tile([128, 512], BF16, name="zeros_b")

        make_identity(nc, ident)
        nc.gpsimd.memset(ones_b, 1.0)
        nc.gpsimd.memset(ones_f, 1.0)
        nc.gpsimd.memset(zeros_b, 0.0)

        # ---- load inputs ----
        nc.sync.dma_start(
            x_sb.rearrange("p (n c) -> p n c", n=NT),
            x_d[:].rearrange("(n p) c -> p n c", p=128),
        )
        nc.sync.dma_start(
            wa_f.rearrange("p (k m) -> p k m", k=2),
            wa_d[:].rearrange("(k p) m -> p k m", p=128),
        )
        nc.sync.dma_start(
            wp_f.rearrange("p (k m) -> p k m", k=2),
            wp_d[:].rearrange("(k p) m -> p k m", p=128),
        )
        nc.vector.tensor_copy(wa_b[:], wa_f[:])
        nc.vector.tensor_copy(wp_b[:], wp_f[:])

        # ---- setup phase: transpose x, compute qT/kT/v ----
        with tc.tile_pool(name="pset", bufs=1, space="PSUM") as pset:
            # x transpose: 32 [128,128] PE transposes, batched 4 per psum bank
            for kc in range(2):
                for ng in range(4):
                    tp = pset.tile([128, 512], F32, tag="tp", bufs=2)
                    for j in range(4):
                        n = ng * 4 + j
                        nc.tensor.transpose(
                            tp[:, j * 128:(j + 1) * 128],
                            x_sb[:, n * 256 + kc * 128: n * 256 + kc * 128 + 128],
                            ident,
                        )
                    nc.vector.tensor_copy(
                        xT[:, kc * T + ng * 512: kc * T + ng * 512 + 512], tp[:]
                    )
            # qT, kT: feature-half fh covers heads (2fh, 2fh+1)
            for fh in range(2):
                for nb in range(NQB):
                    rhs = xT[:, 0 * T + nb * 512: 0 * T + nb * 512 + 512]
                    rhs1 = xT[:, 1 * T + nb * 512: 1 * T + nb * 512 + 512]
                    ps_q = pset.tile([128, 512], F32, tag="mm", bufs=2)
                    nc.tensor.matmul(
                        ps_q, wa_b[:, 0 * 768 + fh * 128: 0 * 768 + fh * 128 + 128],
                        rhs, start=True, stop=False,
                    )
                    nc.tensor.matmul(
                        ps_q, wa_b[:, 1 * 768 + fh * 128: 1 * 768 + fh * 128 + 128],
                        rhs1, start=False, stop=True,
                    )
                    nc.scalar.activation(
                        qT[:, fh * T + nb * 512: fh * T + nb * 512 + 512], ps_q,
                        mybir.ActivationFunctionType.Copy, scale=QSCALE,
                    )
                    ps_k = pset.tile([128, 512], F32, tag="mm", bufs=2)
                    nc.tensor.matmul(
                        ps_k,
                        wa_b[:, 0 * 768 + 256 + fh * 128: 0 * 768 + 256 + fh * 128 + 128],
                        rhs, start=True, stop=False,
                    )
                    nc.tensor.matmul(
                        ps_k,
                        wa_b[:, 1 * 768 + 256 + fh * 128: 1 * 768 + 256 + fh * 128 + 128],
                        rhs1, start=False, stop=True,
                    )
                    nc.scalar.activation(
                        kT[:, fh * T + nb * 512: fh * T + nb * 512 + 512], ps_k,
                        mybir.ActivationFunctionType.Copy,
                    )
            # v (untransposed): v[t, c] for t-tile n
            for n in range(NT):
                ps_v = pset.tile([128, 256], F32, tag="mm", bufs=2)
                for kc in range(2):
                    nc.tensor.matmul(
                        ps_v,
                        xT[:, kc * T + n * 128: kc * T + n * 128 + 128],
                        wa_b[:, kc * 768 + 512: kc * 768 + 768],
                        start=(kc == 0),
                        stop=(kc == 1),
                    )
                nc.vector.tensor_copy(v_sb[:, n * 256: n * 256 + 256], ps_v)

        # ---- attention ----
        def normalize_round(oacc, sums, hp, tqb, pat):
            """Normalize O^T by 1/rowsums and write to yT (deferred one
            round so the bcps matmul never blocks the PE queue)."""
            recip = sb.tile([128, 512], F32, tag="recip", bufs=2, name="recip")
            bcast = sb.tile([128, 512], F32, tag="bcast", bufs=2, name="bcast")
            nc.vector.reciprocal(recip[0:65, :], sums[0:65, :])
            bcps = pat.tile([128, 512], F32, tag="sums", bufs=1, name="bcps")
            nc.tensor.matmul(
                bcps[0:64, :], ones_f[0:1, :], recip[0:1, :],
                start=True, stop=True,
            )
            nc.tensor.matmul(
                bcps[64:128, :], ones_f[64:65, :], recip[64:65, :],
                start=True, stop=True,
            )
            nc.vector.tensor_copy(bcast[:], bcps[:])
            nc.vector.tensor_mul(
                yT[:, hp * T + tqb * 512: hp * T + tqb * 512 + 512],
                oacc[:], bcast[:],
            )

        with tc.tile_pool(name="pat", bufs=1, space="PSUM") as pat:
            prev_round = None
            for hp in range(2):          # head pair: global heads (2hp, 2hp+1)
                for tqb in range(NQB):
                    ntk = 4 * (tqb + 1)
                    tiles = [(h, tk) for tk in range(ntk) for h in range(2)]
                    groups = [
                        tiles[i: i + EXP_GROUP]
                        for i in range(0, len(tiles), EXP_GROUP)
                    ]
                    oacc = sums = None
                    n_pv = 0
                    dbg_col = 0
                    for gi, grp in enumerate(groups):
                        gw = 512 * len(grp)
                        sg = pat.tile([128, gw], F32, tag="sg", bufs=2)
                        pg = sb.tile([128, gw], BF16, tag="P", bufs=4, name="pg")
                        for j, (h, tk) in enumerate(grp):
                            nc.tensor.matmul(
                                sg[:, j * 512:(j + 1) * 512],
                                kT[64 * h: 64 * h + 64,
                                   hp * T + tk * 128: hp * T + tk * 128 + 128],
                                qT[64 * h: 64 * h + 64,
                                   hp * T + tqb * 512: hp * T + tqb * 512 + 512],
                                start=True, stop=True,
                            )
                        # P = 2^(S^T)  (scores already in log2 units)
                        nc.scalar.activation(
                            pg[:], sg[:], mybir.ActivationFunctionType.Exp, scale=LN2
                        )
                        for j, (h, tk) in enumerate(grp):
                            if tk >= 4 * tqb:  # diagonal tile: zero the
                                # triangle (cols below off are skipped by
                                # the off-sliced PV/sums matmuls)
                                off = (tk - 4 * tqb) * 128
                                nc.gpsimd.affine_select(
                                    out=pg[:, j * 512 + off: j * 512 + off + 128],
                                    in_=pg[:, j * 512 + off: j * 512 + off + 128],
                                    compare_op=mybir.AluOpType.is_ge,
                                    fill=0.0,
                                    base=0,
                                    pattern=[[1, 128]],
                                    channel_multiplier=-1,
                                )
                        if gi == 0:
                            # normalize the previous round now: its bcps
                            # matmul is data-ready, so the PE queue never
                            # stalls on it; then allocate this round's
                            # accumulators (tag order keeps WAR deps sound)
                            if prev_round is not None:
                                normalize_round(*prev_round, pat)
                                prev_round = None
                            oacc = pat.tile([128, 512], F32, tag="oacc", bufs=1)
                            sums = pat.tile([128, 512], F32, tag="sums", bufs=1)
                            # zero the accumulator banks (sets every
                            # has_written bit -> interleaved per-region
                            # matmuls below are order-free)
                            nc.tensor.matmul(
                                oacc[:], zeros_b[0:1, 0:128], zeros_b[0:1, :],
                                start=True, stop=False, skip_group_check=True,
                            )
                            nc.tensor.matmul(
                                sums[:], zeros_b[0:1, 0:128], zeros_b[0:1, :],
                                start=True, stop=False, skip_group_check=True,
                            )
                        # PV (head pairs adjacent -> concurrent via col
                        # groups), then the M=1 sums pairs; diagonal tiles
                        # skip their fully-masked leading columns
                        for j, (h, tk) in enumerate(grp):
                            gh = 2 * hp + h
                            off = (tk - 4 * tqb) * 128 if tk >= 4 * tqb else 0
                            n_pv += 1
                            nc.tensor.matmul(
                                oacc[64 * h: 64 * h + 64, off:],
                                v_sb[:, tk * 256 + gh * 64: tk * 256 + gh * 64 + 64],
                                pg[:, j * 512 + off:(j + 1) * 512],
                                start=False, stop=(n_pv == len(tiles)),
                                skip_group_check=True,
                            )
                        for j, (h, tk) in enumerate(grp):
                            off = (tk - 4 * tqb) * 128 if tk >= 4 * tqb else 0
                            nc.tensor.matmul(
                                sums[64 * h: 64 * h + 1, off:],
                                ones_b[:],
                                pg[:, j * 512 + off:(j + 1) * 512],
                                start=False, stop=(n_pv == len(tiles) and j == len(grp) - 1),
                                skip_group_check=True,
                            )
                        if dbg and hp == 0 and tqb == 0:
                            dpt = sb.tile([128, 1536], F32, tag="dbgp", bufs=2, name="dpt")
                            nc.vector.tensor_copy(dpt[:, :gw], pg[:])
                            nc.sync.dma_start(dbg_p[:, dbg_col: dbg_col + gw], dpt[:, :gw])
                            dbg_col += gw
                    if dbg and hp == 0 and tqb == 0:
                        dtile = sb.tile([128, 512], F32, tag="dbgt", bufs=2, name="dtile")
                        nc.vector.tensor_copy(dtile, oacc[:])
                        nc.sync.dma_start(dbg_o[:], dtile)
                        dtile2 = sb.tile([128, 512], F32, tag="dbgt", bufs=2, name="dtile2")
                        nc.vector.tensor_copy(dtile2, sums[:])
                        nc.sync.dma_start(dbg_s[:], dtile2)
                    prev_round = (oacc, sums, hp, tqb)
            normalize_round(*prev_round, pat)

        # ---- output projection ----
        with tc.tile_pool(name="ppr", bufs=1, space="PSUM") as ppr:
            for n in range(NT):
                psz = ppr.tile([128, 256], F32, tag="mm2", bufs=3)
                for fh in range(2):
                    nc.tensor.matmul(
                        psz,
                        yT[:, fh * T + n * 128: fh * T + n * 128 + 128],
                        wp_b[:, fh * 256: fh * 256 + 256],
                        start=(fh == 0),
                        stop=(fh == 1),
                    )
                z_sb = sb.tile([128, 256], F32, tag="z", bufs=3, name="z_sb")
                nc.vector.tensor_copy(z_sb, psz)
                nc.sync.dma_start(
                    y_d[:].rearrange("(n p) c -> p n c", p=128)[:, n: n + 1],
                    z_sb.rearrange("p (n c) -> p n c", n=1),
                )
        sb.release()
    nc.compile()
    return nc


def _get_nc():
    global _cached_nc
    if _cached_nc is None:
        _cached_nc = _build()
    return _cached_nc


def kernel(**inputs):
    from concourse.bass_utils import run_bass_kernel_spmd

    x = np.ascontiguousarray(np.asarray(inputs["x"], dtype=np.float32))
    wa = np.ascontiguousarray(np.asarray(inputs["W_attn"], dtype=np.float32))
    wp = np.ascontiguousarray(np.asarray(inputs["W_proj"], dtype=np.float32))
    nc = _get_nc()
    in_maps = [
        {"x": np.ascontiguousarray(x[b]), "W_attn": wa, "W_proj": wp}
        for b in range(B)
    ]
    res = run_bass_kernel_spmd(nc, in_maps, core_ids=list(range(B)))
    return np.stack([res.results[b]["y"] for b in range(B)], axis=0)
